# revision 1
# baseline (speedup 1.0000x reference)
"""Trainium2 Bass kernel for nn_CmxuLayer: y = U.T @ X, U = 6x6 complex unitary
built from 36 phases, X = [6, 2097152] complex64 given as separate re/im f32 planes.

Strategy (pure data parallel over 8 NeuronCores):
  - Host builds the 6x6 unitary U from the phases (negligible), and packs it into a
    real [96, 96] stationary matrix W implementing the complex matmul on 8
    batch-groups at once (96 = 12 re/im channel components x 8 groups).
  - Each core gets a contiguous batch shard of 262144 columns, reshaped to
    8 groups x 32768. The moving operand is [96, N] f32 in SBUF:
    partitions 0..47  = re channels (c*8+g), partitions 48..95 = im channels.
  - One fp32 PE matmul per 512 columns -> PSUM [96, 512]; DVE/ACT copy to SBUF;
    DMA out as separate re/im planes. Host re-assembles complex64 on gather.
"""

import numpy as np

N_CH = 6
BATCH = 2097152
N_CORES = 8
B_CORE = BATCH // N_CORES      # 262144
G = 8                          # batch groups per core (packed in partition dim)
NG = B_CORE // G               # 32768 columns per group
TILE_N = 512                   # matmul free dim (one PSUM bank @ fp32)
ST = 8192                      # per-group columns per super-tile (DMA granularity)
N_ST = NG // ST                # super-tiles per core

_CACHE = {}


def _build_unitary(mzi_phases, output_phases):
    """Mirror reference.build_unitary in numpy (f32/c64 arithmetic)."""
    n = N_CH
    U = np.eye(n, dtype=np.complex64)
    idx = 0
    mz = np.asarray(mzi_phases, np.float32)
    op = np.asarray(output_phases, np.float32)
    j1 = np.complex64(1j)
    for i in range(n):
        for j in range(i + 1, n):
            theta = mz[idx]
            phi = mz[idx + 1]
            idx += 2
            c = np.complex64(np.cos(theta))
            s = np.complex64(np.sin(theta))
            eip = np.exp(j1 * phi).astype(np.complex64)
            row_i = eip * c * U[i] + s * U[j]
            row_j = -eip * s * U[i] + c * U[j]
            U = U.copy()
            U[i] = row_i
            U[j] = row_j
    U = np.exp(j1 * op)[:, None].astype(np.complex64) * U
    return U


def _build_weights(U):
    """Pack U into the [96, 96] f32 stationary lhsT.

    matmul computes out[m, n] = sum_k lhsT[k, m] * rhs[k, n].
    rhs partition k = ci*8 + g holds xr[ci] of group g (ci in 0..5),
                 k = (6+ci)*8 + g holds xi[ci] of group g.
    out partition m = c*8 + g is y_re[c] of group g,
                  m = (6+c)*8 + g is y_im[c] of group g.
    y = U.T x  =>  y[c] = sum_ci U[ci, c] x[ci].
    """
    Ur = np.ascontiguousarray(U.real.astype(np.float32))
    Ui = np.ascontiguousarray(U.imag.astype(np.float32))
    W = np.zeros((96, 96), np.float32)
    for g in range(G):
        for ci in range(N_CH):
            for c in range(N_CH):
                W[ci * 8 + g, c * 8 + g] = Ur[ci, c]
                W[(6 + ci) * 8 + g, c * 8 + g] = -Ui[ci, c]
                W[ci * 8 + g, (6 + c) * 8 + g] = Ui[ci, c]
                W[(6 + ci) * 8 + g, (6 + c) * 8 + g] = Ur[ci, c]
    return W


def _get_compiled():
    if "nc" in _CACHE:
        return _CACHE["nc"]

    import concourse.bass as bass
    import concourse.mybir as mybir
    from concourse import bacc
    from concourse.bass import ds, ts
    from concourse.tile import TileContext

    f32 = mybir.dt.float32
    nc = bacc.Bacc(
        trn_type="TRN2",
        target_bir_lowering=False,
        debug=False,
        num_devices=N_CORES,
    )
    xr = nc.dram_tensor("xr", [N_CH, B_CORE], f32, kind="ExternalInput").ap()
    xi = nc.dram_tensor("xi", [N_CH, B_CORE], f32, kind="ExternalInput").ap()
    w = nc.dram_tensor("w", [96, 96], f32, kind="ExternalInput").ap()
    yre = nc.dram_tensor("yre", [N_CH, B_CORE], f32, kind="ExternalOutput").ap()
    yim = nc.dram_tensor("yim", [N_CH, B_CORE], f32, kind="ExternalOutput").ap()

    xr_r = xr.rearrange("c (g n) -> c g n", g=G)
    xi_r = xi.rearrange("c (g n) -> c g n", g=G)
    yre_r = yre.rearrange("c (g n) -> c g n", g=G)
    yim_r = yim.rearrange("c (g n) -> c g n", g=G)

    with TileContext(nc) as tc:
        with (
            tc.tile_pool(name="wpool", bufs=1) as wp,
            tc.tile_pool(name="mv", bufs=2) as mvp,
            tc.tile_pool(name="ot", bufs=2) as op,
            tc.tile_pool(name="ps", bufs=8, space="PSUM") as pp,
        ):
            wt = wp.tile([96, 96], f32)
            nc.sync.dma_start(out=wt[:], in_=w[:])
            for s in range(N_ST):
                mv = mvp.tile([96, ST], f32)
                nc.sync.dma_start(out=mv[0:48, :], in_=xr_r[:, :, ds(s * ST, ST)])
                nc.sync.dma_start(out=mv[48:96, :], in_=xi_r[:, :, ds(s * ST, ST)])
                ot = op.tile([96, ST], f32)
                for j in range(ST // TILE_N):
                    ps = pp.tile([96, TILE_N], f32)
                    nc.tensor.matmul(
                        out=ps[:],
                        lhsT=wt[:],
                        rhs=mv[:, ts(j, TILE_N)],
                        start=True,
                        stop=True,
                    )
                    if j % 2 == 0:
                        nc.vector.tensor_copy(out=ot[:, ts(j, TILE_N)], in_=ps[:])
                    else:
                        nc.scalar.copy(out=ot[:, ts(j, TILE_N)], in_=ps[:])
                nc.sync.dma_start(out=yre_r[:, :, ds(s * ST, ST)], in_=ot[0:48, :])
                nc.sync.dma_start(out=yim_r[:, :, ds(s * ST, ST)], in_=ot[48:96, :])

    nc.compile()
    _CACHE["nc"] = nc
    return nc


def kernel(field_re, field_im, mzi_phases, output_phases):
    from concourse import bass_utils

    field_re = np.ascontiguousarray(np.asarray(field_re), dtype=np.float32)
    field_im = np.ascontiguousarray(np.asarray(field_im), dtype=np.float32)
    U = _build_unitary(mzi_phases, output_phases)
    W = _build_weights(U)

    nc = _get_compiled()
    in_maps = []
    for i in range(N_CORES):
        sl = slice(i * B_CORE, (i + 1) * B_CORE)
        in_maps.append(
            {
                "xr": np.ascontiguousarray(field_re[:, sl]),
                "xi": np.ascontiguousarray(field_im[:, sl]),
                "w": W,
            }
        )
    res = bass_utils.run_bass_kernel_spmd(nc, in_maps, core_ids=list(range(N_CORES)))

    out = np.empty((N_CH, BATCH), np.complex64)
    for i in range(N_CORES):
        sl = slice(i * B_CORE, (i + 1) * B_CORE)
        out.real[:, sl] = res.results[i]["yre"]
        out.imag[:, sl] = res.results[i]["yim"]
    return out


# revision 7
# speedup vs baseline: 1.0302x; 1.0302x over previous
"""Trainium2 Bass kernel for nn_CmxuLayer: y = U.T @ X, U = 6x6 complex unitary
built from 36 phases, X = [6, 2097152] complex64 given as separate re/im f32 planes.

Strategy (pure data parallel over 8 NeuronCores):
  - Host builds the 6x6 unitary U from the phases (negligible), and packs it into a
    real [96, 96] stationary matrix W implementing the complex matmul on 8
    batch-groups at once (96 = 12 re/im channel components x 8 groups).
  - Each core gets a contiguous batch shard of 262144 columns, reshaped to
    8 groups x 32768. The moving operand is [96, N] f32 in SBUF:
    partitions 0..47  = re channels (c*8+g), partitions 48..95 = im channels.
  - One fp32 PE matmul per 512 columns -> PSUM [96, 512]; DVE/ACT copy to SBUF;
    DMA out as separate re/im planes. Host re-assembles complex64 on gather.
"""

import numpy as np

N_CH = 6
BATCH = 2097152
N_CORES = 8
B_CORE = BATCH // N_CORES      # 262144
G = 8                          # batch groups per core (packed in partition dim)
NG = B_CORE // G               # 32768 columns per group
TILE_N = 512                   # matmul free dim (one PSUM bank @ fp32)
ST = 4096                      # per-group columns per super-tile (DMA granularity)
N_ST = NG // ST                # super-tiles per core
USE_F32R = True                # fp32r matmul: 1 cyc/col vs fp32's 4 (reduced precision)

_CACHE = {}


def _build_unitary(mzi_phases, output_phases):
    """Mirror reference.build_unitary in numpy (f32/c64 arithmetic)."""
    n = N_CH
    U = np.eye(n, dtype=np.complex64)
    idx = 0
    mz = np.asarray(mzi_phases, np.float32)
    op = np.asarray(output_phases, np.float32)
    j1 = np.complex64(1j)
    for i in range(n):
        for j in range(i + 1, n):
            theta = mz[idx]
            phi = mz[idx + 1]
            idx += 2
            c = np.complex64(np.cos(theta))
            s = np.complex64(np.sin(theta))
            eip = np.exp(j1 * phi).astype(np.complex64)
            row_i = eip * c * U[i] + s * U[j]
            row_j = -eip * s * U[i] + c * U[j]
            U = U.copy()
            U[i] = row_i
            U[j] = row_j
    U = np.exp(j1 * op)[:, None].astype(np.complex64) * U
    return U


def _build_weights(U):
    """Pack U into the [96, 96] f32 stationary lhsT.

    matmul computes out[m, n] = sum_k lhsT[k, m] * rhs[k, n].
    rhs partition k = ci*8 + g holds xr[ci] of group g (ci in 0..5),
                 k = (6+ci)*8 + g holds xi[ci] of group g.
    out partition m = c*8 + g is y_re[c] of group g,
                  m = (6+c)*8 + g is y_im[c] of group g.
    y = U.T x  =>  y[c] = sum_ci U[ci, c] x[ci].
    """
    Ur = np.ascontiguousarray(U.real.astype(np.float32))
    Ui = np.ascontiguousarray(U.imag.astype(np.float32))
    W = np.zeros((96, 96), np.float32)
    for g in range(G):
        for ci in range(N_CH):
            for c in range(N_CH):
                W[ci * 8 + g, c * 8 + g] = Ur[ci, c]
                W[(6 + ci) * 8 + g, c * 8 + g] = -Ui[ci, c]
                W[ci * 8 + g, (6 + c) * 8 + g] = Ui[ci, c]
                W[(6 + ci) * 8 + g, (6 + c) * 8 + g] = Ur[ci, c]
    return W


def _get_compiled(reps=1):
    key = ("nc", reps)
    if key in _CACHE:
        return _CACHE[key]

    import concourse.bass as bass
    import concourse.mybir as mybir
    from concourse import bacc
    from concourse.bass import ds, ts
    from concourse.tile import TileContext

    f32 = mybir.dt.float32
    f32r = mybir.dt.float32r
    in_dt = f32r if USE_F32R else f32
    nc = bacc.Bacc(
        trn_type="TRN2",
        target_bir_lowering=False,
        debug=False,
        num_devices=N_CORES,
    )
    xr = nc.dram_tensor("xr", [N_CH, B_CORE], in_dt, kind="ExternalInput").ap()
    xi = nc.dram_tensor("xi", [N_CH, B_CORE], in_dt, kind="ExternalInput").ap()
    w = nc.dram_tensor("w", [96, 96], in_dt, kind="ExternalInput").ap()
    yre = nc.dram_tensor("yre", [N_CH, B_CORE], f32, kind="ExternalOutput").ap()
    yim = nc.dram_tensor("yim", [N_CH, B_CORE], f32, kind="ExternalOutput").ap()

    xr_r = xr.rearrange("c (g n) -> c g n", g=G)
    xi_r = xi.rearrange("c (g n) -> c g n", g=G)
    yre_r = yre.rearrange("c (g n) -> c g n", g=G)
    yim_r = yim.rearrange("c (g n) -> c g n", g=G)

    with TileContext(nc) as tc:
        with (
            tc.tile_pool(name="wpool", bufs=1) as wp,
            tc.tile_pool(name="mv", bufs=3) as mvp,
            tc.tile_pool(name="ot", bufs=3) as op,
            tc.tile_pool(name="ps", bufs=8, space="PSUM") as pp,
        ):
            wt = wp.tile([96, 96], in_dt)
            nc.sync.dma_start(out=wt[:], in_=w[:])

            def body():
                for s in range(N_ST):
                    mv = mvp.tile([96, ST], in_dt, tag="mv")
                    nc.sync.dma_start(out=mv[0:48, :], in_=xr_r[:, :, ds(s * ST, ST)])
                    nc.sync.dma_start(out=mv[48:96, :], in_=xi_r[:, :, ds(s * ST, ST)])
                    ot = op.tile([96, ST], f32, tag="ot")
                    for j in range(ST // TILE_N):
                        ps = pp.tile([96, TILE_N], f32, tag="ps")
                        nc.tensor.matmul(
                            out=ps[:],
                            lhsT=wt[:],
                            rhs=mv[:, ts(j, TILE_N)],
                            start=True,
                            stop=True,
                        )
                        if j % 2 == 0:
                            nc.vector.tensor_copy(out=ot[:, ts(j, TILE_N)], in_=ps[:])
                        else:
                            nc.scalar.copy(out=ot[:, ts(j, TILE_N)], in_=ps[:])
                    nc.sync.dma_start(out=yre_r[:, :, ds(s * ST, ST)], in_=ot[0:48, :])
                    nc.sync.dma_start(out=yim_r[:, :, ds(s * ST, ST)], in_=ot[48:96, :])

            if reps == 1:
                body()
            else:
                with tc.For_i(0, reps, 1):
                    body()

    nc.compile()
    _CACHE[key] = nc
    return nc


def kernel(field_re, field_im, mzi_phases, output_phases):
    from concourse import bass_utils

    field_re = np.ascontiguousarray(np.asarray(field_re), dtype=np.float32)
    field_im = np.ascontiguousarray(np.asarray(field_im), dtype=np.float32)
    U = _build_unitary(mzi_phases, output_phases)
    W = _build_weights(U)

    nc = _get_compiled()
    in_maps = []
    for i in range(N_CORES):
        sl = slice(i * B_CORE, (i + 1) * B_CORE)
        in_maps.append(
            {
                "xr": np.ascontiguousarray(field_re[:, sl]),
                "xi": np.ascontiguousarray(field_im[:, sl]),
                "w": W,
            }
        )
    res = bass_utils.run_bass_kernel_spmd(nc, in_maps, core_ids=list(range(N_CORES)))

    out = np.empty((N_CH, BATCH), np.complex64)
    for i in range(N_CORES):
        sl = slice(i * B_CORE, (i + 1) * B_CORE)
        out.real[:, sl] = res.results[i]["yre"]
        out.imag[:, sl] = res.results[i]["yim"]
    return out


# revision 15
# speedup vs baseline: 71.9370x; 69.8294x over previous
"""Trainium2 Bass kernel for nn_CmxuLayer: y = U.T @ X, U = 6x6 complex unitary
built from 36 phases, X = [6, 2097152] complex64 given as separate re/im f32 planes.

Strategy (pure data parallel over 8 NeuronCores):
  - Host builds the 6x6 unitary U from the phases (negligible), and packs it into a
    real [120, 120] stationary matrix W implementing the complex matmul on 10
    batch-groups at once (120 = 12 re/im channel components x 10 groups).
  - Each core gets a contiguous batch shard of 262144 columns, zero-padded to
    266240 and reshaped to 10 groups x 26624. The moving operand is [120, N] f32
    in SBUF: partitions 0..59 = re channels (c*10+g), 60..119 = im channels.
    120 partitions balance the SBUF<->DMA port swizzle (96 would leave the even
    SDMA engines carrying 2x the bytes of the odd ones).
  - One fp32r PE matmul per 512 columns -> PSUM [120, 512]; DVE/ACT copy to SBUF;
    DMA out as separate re/im planes. Host re-assembles complex64 on gather.
  - Input DMAs issue on the SP HWDGE ring, output DMAs on the ACT ring, so a
    tile's output DMA never head-of-line-blocks the next tile's input DMAs.
"""

import numpy as np

N_CH = 6
BATCH = 2097152
N_CORES = 8
B_CORE = BATCH // N_CORES      # 262144 true columns per core
G = 10                         # batch groups per core (packed in partition dim)
NG = 26624                     # padded columns per group (13 * 2048)
B_PAD = G * NG                 # 266240 padded columns per core
K = 12 * G                     # 120 partitions
TILE_N = 512                   # matmul free dim (one PSUM bank @ fp32)
ST = 2048                      # per-group columns per super-tile (DMA granularity)
N_ST = NG // ST                # 13 super-tiles per core
USE_F32R = False                # fp32r matmul: 1 cyc/col vs fp32's 4 (reduced precision)

_CACHE = {}


def _build_unitary(mzi_phases, output_phases):
    """Mirror reference.build_unitary in numpy (f32/c64 arithmetic)."""
    n = N_CH
    U = np.eye(n, dtype=np.complex64)
    idx = 0
    mz = np.asarray(mzi_phases, np.float32)
    op = np.asarray(output_phases, np.float32)
    j1 = np.complex64(1j)
    for i in range(n):
        for j in range(i + 1, n):
            theta = mz[idx]
            phi = mz[idx + 1]
            idx += 2
            c = np.complex64(np.cos(theta))
            s = np.complex64(np.sin(theta))
            eip = np.exp(j1 * phi).astype(np.complex64)
            row_i = eip * c * U[i] + s * U[j]
            row_j = -eip * s * U[i] + c * U[j]
            U = U.copy()
            U[i] = row_i
            U[j] = row_j
    U = np.exp(j1 * op)[:, None].astype(np.complex64) * U
    return U


def _build_weights(U):
    """Pack U into the [K, K] f32 stationary lhsT.

    matmul computes out[m, n] = sum_k lhsT[k, m] * rhs[k, n].
    rhs partition k = ci*G + g holds xr[ci] of group g (ci in 0..5),
                 k = (6+ci)*G + g holds xi[ci] of group g.
    out partition m = c*G + g is y_re[c] of group g,
                  m = (6+c)*G + g is y_im[c] of group g.
    y = U.T x  =>  y[c] = sum_ci U[ci, c] x[ci].
    """
    Ur = np.ascontiguousarray(U.real.astype(np.float32))
    Ui = np.ascontiguousarray(U.imag.astype(np.float32))
    W = np.zeros((K, K), np.float32)
    for g in range(G):
        for ci in range(N_CH):
            for c in range(N_CH):
                W[ci * G + g, c * G + g] = Ur[ci, c]
                W[(6 + ci) * G + g, c * G + g] = -Ui[ci, c]
                W[ci * G + g, (6 + c) * G + g] = Ui[ci, c]
                W[(6 + ci) * G + g, (6 + c) * G + g] = Ur[ci, c]
    return W


def _get_compiled(reps=1, variant="full", f32r=None):
    if f32r is None:
        f32r = USE_F32R
    key = ("nc", reps, variant, f32r)
    if key in _CACHE:
        return _CACHE[key]

    import concourse.bass as bass
    import concourse.mybir as mybir
    from concourse import bacc
    from concourse.bass import ds, ts
    from concourse.tile import TileContext

    f32 = mybir.dt.float32
    in_dt = mybir.dt.float32r if f32r else f32
    nc = bacc.Bacc(
        trn_type="TRN2",
        target_bir_lowering=False,
        debug=False,
        num_devices=N_CORES,
    )
    H = K // 2  # 60: partition split between re and im halves
    xr = nc.dram_tensor("xr", [N_CH, B_PAD], in_dt, kind="ExternalInput").ap()
    xi = nc.dram_tensor("xi", [N_CH, B_PAD], in_dt, kind="ExternalInput").ap()
    w = nc.dram_tensor("w", [K, K], in_dt, kind="ExternalInput").ap()
    yre = nc.dram_tensor("yre", [N_CH, B_PAD], f32, kind="ExternalOutput").ap()
    yim = nc.dram_tensor("yim", [N_CH, B_PAD], f32, kind="ExternalOutput").ap()

    xr_r = xr.rearrange("c (g n) -> c g n", g=G)
    xi_r = xi.rearrange("c (g n) -> c g n", g=G)
    yre_r = yre.rearrange("c (g n) -> c g n", g=G)
    yim_r = yim.rearrange("c (g n) -> c g n", g=G)

    with TileContext(nc) as tc:
        with (
            tc.tile_pool(name="wpool", bufs=1) as wp,
            tc.tile_pool(name="mv", bufs=4) as mvp,
            tc.tile_pool(name="ot", bufs=4) as op,
            tc.tile_pool(name="ps", bufs=8, space="PSUM") as pp,
        ):
            wt = wp.tile([K, K], in_dt)
            nc.sync.dma_start(out=wt[:], in_=w[:])

            def body():
                for s in range(N_ST):
                    mv = mvp.tile([K, ST], in_dt, tag="mv")
                    nc.sync.dma_start(out=mv[0:H, :], in_=xr_r[:, :, ds(s * ST, ST)])
                    nc.sync.dma_start(out=mv[H:K, :], in_=xi_r[:, :, ds(s * ST, ST)])
                    if variant == "dma":
                        # stream straight back out, skipping compute
                        nc.scalar.dma_start(
                            out=yre_r[:, :, ds(s * ST, ST)], in_=mv[0:H, :].bitcast(f32)
                        )
                        nc.scalar.dma_start(
                            out=yim_r[:, :, ds(s * ST, ST)], in_=mv[H:K, :].bitcast(f32)
                        )
                        continue
                    ot = op.tile([K, ST], f32, tag="ot")
                    for j in range(ST // TILE_N):
                        ps = pp.tile([K, TILE_N], f32, tag="ps")
                        nc.tensor.matmul(
                            out=ps[:],
                            lhsT=wt[:],
                            rhs=mv[:, ts(j, TILE_N)],
                            start=True,
                            stop=True,
                        )
                        if j % 2 == 0:
                            nc.vector.tensor_copy(out=ot[:, ts(j, TILE_N)], in_=ps[:])
                        else:
                            nc.scalar.copy(out=ot[:, ts(j, TILE_N)], in_=ps[:])
                    if variant == "nooutdma":
                        continue
                    # Output DMAs on the ACT HWDGE ring so they don't
                    # head-of-line-block the next tile's input DMAs on SP's ring.
                    nc.scalar.dma_start(out=yre_r[:, :, ds(s * ST, ST)], in_=ot[0:H, :])
                    nc.scalar.dma_start(out=yim_r[:, :, ds(s * ST, ST)], in_=ot[H:K, :])

            if reps == 1:
                body()
            else:
                with tc.For_i(0, reps, 1):
                    body()

    nc.compile()
    _CACHE[key] = nc
    return nc


def _pad_shard(plane, sl):
    out = np.zeros((N_CH, B_PAD), np.float32)
    out[:, :B_CORE] = plane[:, sl]
    return out


def kernel(field_re, field_im, mzi_phases, output_phases):
    from concourse import bass_utils

    field_re = np.asarray(field_re)
    field_im = np.asarray(field_im)
    U = _build_unitary(mzi_phases, output_phases)
    W = _build_weights(U)

    nc = _get_compiled()
    in_maps = []
    for i in range(N_CORES):
        sl = slice(i * B_CORE, (i + 1) * B_CORE)
        in_maps.append(
            {
                "xr": _pad_shard(field_re, sl),
                "xi": _pad_shard(field_im, sl),
                "w": W,
            }
        )
    res = bass_utils.run_bass_kernel_spmd(nc, in_maps, core_ids=list(range(N_CORES)))

    out = np.empty((N_CH, BATCH), np.complex64)
    for i in range(N_CORES):
        sl = slice(i * B_CORE, (i + 1) * B_CORE)
        out.real[:, sl] = res.results[i]["yre"][:, :B_CORE]
        out.imag[:, sl] = res.results[i]["yim"][:, :B_CORE]
    return out


# revision 18
# speedup vs baseline: 74.3730x; 1.0339x over previous
"""Trainium2 Bass kernel for nn_CmxuLayer: y = U.T @ X, U = 6x6 complex unitary
built from 36 phases, X = [6, 2097152] complex64 given as separate re/im f32 planes.

Strategy (pure data parallel over 8 NeuronCores):
  - Host builds the 6x6 unitary U from the phases (negligible), and packs it into a
    real [120, 120] stationary matrix W implementing the complex matmul on 10
    batch-groups at once (120 = 12 re/im channel components x 10 groups).
  - Each core gets a contiguous batch shard of 262144 columns, zero-padded to
    266240 and reshaped to 10 groups x 26624. The moving operand is [120, N] f32
    in SBUF: partitions 0..59 = re channels (c*10+g), 60..119 = im channels.
    120 partitions balance the SBUF<->DMA port swizzle (96 would leave the even
    SDMA engines carrying 2x the bytes of the odd ones).
  - One fp32 PE matmul per 512 columns -> PSUM [120, 512]; DVE/ACT copy to SBUF;
    DMA out as separate re/im planes. Host re-assembles complex64 on gather.
    (fp32 matmul streams at 4 cyc/col but still hides under the DMA floor;
    measured <1% slower than the reduced-precision fp32r mode.)
  - Input DMAs issue on the SP HWDGE ring, output DMAs on the ACT ring, so a
    tile's output DMA never head-of-line-blocks the next tile's input DMAs.
"""

import numpy as np

N_CH = 6
BATCH = 2097152
N_CORES = 8
B_CORE = BATCH // N_CORES      # 262144 true columns per core
G = 10                         # batch groups per core (packed in partition dim)
NG = 26624                     # padded columns per group (13 * 2048)
B_PAD = G * NG                 # 266240 padded columns per core
K = 12 * G                     # 120 partitions
TILE_N = 512                   # matmul free dim (one PSUM bank @ fp32)
ST = 2048                      # per-group columns per super-tile (DMA granularity)
N_ST = NG // ST                # 13 super-tiles per core
USE_F32R = False                # fp32r matmul: 1 cyc/col vs fp32's 4 (reduced precision)

_CACHE = {}


def _build_unitary(mzi_phases, output_phases):
    """Mirror reference.build_unitary in numpy (f32/c64 arithmetic)."""
    n = N_CH
    U = np.eye(n, dtype=np.complex64)
    idx = 0
    mz = np.asarray(mzi_phases, np.float32)
    op = np.asarray(output_phases, np.float32)
    j1 = np.complex64(1j)
    for i in range(n):
        for j in range(i + 1, n):
            theta = mz[idx]
            phi = mz[idx + 1]
            idx += 2
            c = np.complex64(np.cos(theta))
            s = np.complex64(np.sin(theta))
            eip = np.exp(j1 * phi).astype(np.complex64)
            row_i = eip * c * U[i] + s * U[j]
            row_j = -eip * s * U[i] + c * U[j]
            U = U.copy()
            U[i] = row_i
            U[j] = row_j
    U = np.exp(j1 * op)[:, None].astype(np.complex64) * U
    return U


def _build_weights(U):
    """Pack U into the [K, K] f32 stationary lhsT.

    matmul computes out[m, n] = sum_k lhsT[k, m] * rhs[k, n].
    rhs partition k = ci*G + g holds xr[ci] of group g (ci in 0..5),
                 k = (6+ci)*G + g holds xi[ci] of group g.
    out partition m = c*G + g is y_re[c] of group g,
                  m = (6+c)*G + g is y_im[c] of group g.
    y = U.T x  =>  y[c] = sum_ci U[ci, c] x[ci].
    """
    Ur = np.ascontiguousarray(U.real.astype(np.float32))
    Ui = np.ascontiguousarray(U.imag.astype(np.float32))
    W = np.zeros((K, K), np.float32)
    for g in range(G):
        for ci in range(N_CH):
            for c in range(N_CH):
                W[ci * G + g, c * G + g] = Ur[ci, c]
                W[(6 + ci) * G + g, c * G + g] = -Ui[ci, c]
                W[ci * G + g, (6 + c) * G + g] = Ui[ci, c]
                W[(6 + ci) * G + g, (6 + c) * G + g] = Ur[ci, c]
    return W


def _get_compiled(reps=1, variant="full", f32r=None):
    if f32r is None:
        f32r = USE_F32R
    key = ("nc", reps, variant, f32r)
    if key in _CACHE:
        return _CACHE[key]

    import concourse.bass as bass
    import concourse.mybir as mybir
    from concourse import bacc
    from concourse.bass import ds, ts
    from concourse.tile import TileContext

    f32 = mybir.dt.float32
    in_dt = mybir.dt.float32r if f32r else f32
    nc = bacc.Bacc(
        trn_type="TRN2",
        target_bir_lowering=False,
        debug=False,
        num_devices=N_CORES,
    )
    H = K // 2  # 60: partition split between re and im halves
    xr = nc.dram_tensor("xr", [N_CH, B_PAD], in_dt, kind="ExternalInput").ap()
    xi = nc.dram_tensor("xi", [N_CH, B_PAD], in_dt, kind="ExternalInput").ap()
    w = nc.dram_tensor("w", [K, K], in_dt, kind="ExternalInput").ap()
    yre = nc.dram_tensor("yre", [N_CH, B_PAD], f32, kind="ExternalOutput").ap()
    yim = nc.dram_tensor("yim", [N_CH, B_PAD], f32, kind="ExternalOutput").ap()

    xr_r = xr.rearrange("c (g n) -> c g n", g=G)
    xi_r = xi.rearrange("c (g n) -> c g n", g=G)
    yre_r = yre.rearrange("c (g n) -> c g n", g=G)
    yim_r = yim.rearrange("c (g n) -> c g n", g=G)

    with TileContext(nc) as tc:
        with (
            tc.tile_pool(name="wpool", bufs=1) as wp,
            tc.tile_pool(name="mv", bufs=4) as mvp,
            tc.tile_pool(name="ot", bufs=4) as op,
            tc.tile_pool(name="ps", bufs=8, space="PSUM") as pp,
        ):
            wt = wp.tile([K, K], in_dt)
            nc.sync.dma_start(out=wt[:], in_=w[:])

            if variant == "big":
                # 6 super-tiles of 4096 cols + 1 of 2048 (fewer, larger DMAs)
                st_list = [(o * 4096, 4096) for o in range(6)] + [(24576, 2048)]
            else:
                st_list = [(s * ST, ST) for s in range(N_ST)]

            def body():
                for off, stn in st_list:
                    mv = mvp.tile([K, stn], in_dt, tag="mv")
                    nc.sync.dma_start(out=mv[0:H, :], in_=xr_r[:, :, ds(off, stn)])
                    nc.sync.dma_start(out=mv[H:K, :], in_=xi_r[:, :, ds(off, stn)])
                    if variant == "dma":
                        # stream straight back out, skipping compute
                        nc.scalar.dma_start(
                            out=yre_r[:, :, ds(off, stn)], in_=mv[0:H, :].bitcast(f32)
                        )
                        nc.scalar.dma_start(
                            out=yim_r[:, :, ds(off, stn)], in_=mv[H:K, :].bitcast(f32)
                        )
                        continue
                    ot = op.tile([K, stn], f32, tag="ot")
                    for j in range(stn // TILE_N):
                        ps = pp.tile([K, TILE_N], f32, tag="ps")
                        nc.tensor.matmul(
                            out=ps[:],
                            lhsT=wt[:],
                            rhs=mv[:, ts(j, TILE_N)],
                            start=True,
                            stop=True,
                        )
                        if j % 2 == 0:
                            nc.vector.tensor_copy(out=ot[:, ts(j, TILE_N)], in_=ps[:])
                        else:
                            nc.scalar.copy(out=ot[:, ts(j, TILE_N)], in_=ps[:])
                    if variant == "nooutdma":
                        continue
                    # Output DMAs on the ACT HWDGE ring so they don't
                    # head-of-line-block the next tile's input DMAs on SP's ring.
                    nc.scalar.dma_start(out=yre_r[:, :, ds(off, stn)], in_=ot[0:H, :])
                    nc.scalar.dma_start(out=yim_r[:, :, ds(off, stn)], in_=ot[H:K, :])

            if reps == 1:
                body()
            else:
                with tc.For_i(0, reps, 1):
                    body()

    nc.compile()
    _CACHE[key] = nc
    return nc


def _pad_shard(plane, sl):
    out = np.zeros((N_CH, B_PAD), np.float32)
    out[:, :B_CORE] = plane[:, sl]
    return out


def kernel(field_re, field_im, mzi_phases, output_phases):
    from concourse import bass_utils

    field_re = np.asarray(field_re)
    field_im = np.asarray(field_im)
    U = _build_unitary(mzi_phases, output_phases)
    W = _build_weights(U)

    nc = _get_compiled()
    in_maps = []
    for i in range(N_CORES):
        sl = slice(i * B_CORE, (i + 1) * B_CORE)
        in_maps.append(
            {
                "xr": _pad_shard(field_re, sl),
                "xi": _pad_shard(field_im, sl),
                "w": W,
            }
        )
    res = bass_utils.run_bass_kernel_spmd(nc, in_maps, core_ids=list(range(N_CORES)))

    out = np.empty((N_CH, BATCH), np.complex64)
    for i in range(N_CORES):
        sl = slice(i * B_CORE, (i + 1) * B_CORE)
        out.real[:, sl] = res.results[i]["yre"][:, :B_CORE]
        out.imag[:, sl] = res.results[i]["yim"][:, :B_CORE]
    return out


# revision 28
# speedup vs baseline: 76.5678x; 1.0295x over previous
"""Trainium2 Bass kernel for nn_CmxuLayer: y = U.T @ X, U = 6x6 complex unitary
built from 36 phases, X = [6, 2097152] complex64 given as separate re/im f32 planes.

Strategy (pure data parallel over 8 NeuronCores):
  - Host builds the 6x6 unitary U from the phases (negligible), and packs it into a
    real [120, 120] stationary matrix W implementing the complex matmul on 10
    batch-groups at once (120 = 12 re/im channel components x 10 groups).
  - Each core gets a contiguous batch shard of 262144 columns, zero-padded to
    266240 and reshaped to 10 groups x 26624. The moving operand is [120, N] f32
    in SBUF: partitions 0..59 = re channels (c*10+g), 60..119 = im channels.
    120 partitions balance the SBUF<->DMA port swizzle (96 would leave the even
    SDMA engines carrying 2x the bytes of the odd ones).
  - One fp32 PE matmul per 512 columns -> PSUM [120, 512]; DVE/ACT copy to SBUF;
    DMA out as separate re/im planes. Host re-assembles complex64 on gather.
    (fp32 matmul streams at 4 cyc/col but still hides under the DMA floor;
    measured <1% slower than the reduced-precision fp32r mode.)
  - Input DMAs issue on the SP HWDGE ring, output DMAs on the ACT ring, so a
    tile's output DMA never head-of-line-blocks the next tile's input DMAs.
"""

import numpy as np

N_CH = 6
BATCH = 2097152
N_CORES = 8
B_CORE = BATCH // N_CORES      # 262144 true columns per core
G = 10                         # batch groups per core (packed in partition dim)
NG = 26624                     # padded columns per group (13 * 2048)
B_PAD = G * NG                 # 266240 padded columns per core
K = 12 * G                     # 120 partitions
TILE_N = 512                   # matmul free dim (one PSUM bank @ fp32)
ST = 2048                      # per-group columns per super-tile (DMA granularity)
N_ST = NG // ST                # 13 super-tiles per core
USE_F32R = False                # fp32r matmul: 1 cyc/col vs fp32's 4 (reduced precision)

_CACHE = {}


def _build_unitary(mzi_phases, output_phases):
    """Mirror reference.build_unitary in numpy (f32/c64 arithmetic)."""
    n = N_CH
    U = np.eye(n, dtype=np.complex64)
    idx = 0
    mz = np.asarray(mzi_phases, np.float32)
    op = np.asarray(output_phases, np.float32)
    j1 = np.complex64(1j)
    for i in range(n):
        for j in range(i + 1, n):
            theta = mz[idx]
            phi = mz[idx + 1]
            idx += 2
            c = np.complex64(np.cos(theta))
            s = np.complex64(np.sin(theta))
            eip = np.exp(j1 * phi).astype(np.complex64)
            row_i = eip * c * U[i] + s * U[j]
            row_j = -eip * s * U[i] + c * U[j]
            U = U.copy()
            U[i] = row_i
            U[j] = row_j
    U = np.exp(j1 * op)[:, None].astype(np.complex64) * U
    return U


def _build_weights(U):
    """Pack U into the [K, K] f32 stationary lhsT.

    matmul computes out[m, n] = sum_k lhsT[k, m] * rhs[k, n].
    rhs partition k = ci*G + g holds xr[ci] of group g (ci in 0..5),
                 k = (6+ci)*G + g holds xi[ci] of group g.
    out partition m = c*G + g is y_re[c] of group g,
                  m = (6+c)*G + g is y_im[c] of group g.
    y = U.T x  =>  y[c] = sum_ci U[ci, c] x[ci].
    """
    Ur = np.ascontiguousarray(U.real.astype(np.float32))
    Ui = np.ascontiguousarray(U.imag.astype(np.float32))
    W = np.zeros((K, K), np.float32)
    for g in range(G):
        for ci in range(N_CH):
            for c in range(N_CH):
                W[ci * G + g, c * G + g] = Ur[ci, c]
                W[(6 + ci) * G + g, c * G + g] = -Ui[ci, c]
                W[ci * G + g, (6 + c) * G + g] = Ui[ci, c]
                W[(6 + ci) * G + g, (6 + c) * G + g] = Ur[ci, c]
    return W


def _get_compiled(reps=1, variant="full", f32r=None):
    if f32r is None:
        f32r = USE_F32R
    key = ("nc", reps, variant, f32r)
    if key in _CACHE:
        return _CACHE[key]

    import concourse.bass as bass
    import concourse.mybir as mybir
    from concourse import bacc
    from concourse.bass import ds, ts
    from concourse.tile import TileContext

    f32 = mybir.dt.float32
    in_dt = mybir.dt.float32r if f32r else f32
    nc = bacc.Bacc(
        trn_type="TRN2",
        target_bir_lowering=False,
        debug=False,
        num_devices=N_CORES,
    )
    H = K // 2  # 60: partition split between re and im halves
    xr = nc.dram_tensor("xr", [N_CH, B_PAD], in_dt, kind="ExternalInput").ap()
    xi = nc.dram_tensor("xi", [N_CH, B_PAD], in_dt, kind="ExternalInput").ap()
    w = nc.dram_tensor("w", [K, K], in_dt, kind="ExternalInput").ap()
    yre = nc.dram_tensor("yre", [N_CH, B_PAD], f32, kind="ExternalOutput").ap()
    yim = nc.dram_tensor("yim", [N_CH, B_PAD], f32, kind="ExternalOutput").ap()

    xr_r = xr.rearrange("c (g n) -> c g n", g=G)
    xi_r = xi.rearrange("c (g n) -> c g n", g=G)
    yre_r = yre.rearrange("c (g n) -> c g n", g=G)
    yim_r = yim.rearrange("c (g n) -> c g n", g=G)

    n_bufs = {"v2": 6, "v2c": 6, "v2ac": 6, "v2bc": 6, "v3c": 8, "v3ac": 8}.get(
        variant, 4
    )
    with TileContext(nc) as tc:
        with (
            tc.tile_pool(name="wpool", bufs=1) as wp,
            tc.tile_pool(name="mv", bufs=n_bufs) as mvp,
            tc.tile_pool(name="ot", bufs=n_bufs) as op,
            tc.tile_pool(name="ps", bufs=8, space="PSUM") as pp,
        ):
            wt = wp.tile([K, K], in_dt)
            if variant in ("v2", "v2a", "v2ac", "v3ac"):
                # SWDGE (gpsimd) ring: keeps the 120 sub-512B weight
                # descriptors off the SP ring ahead of the first input DMAs.
                nc.gpsimd.dma_start(out=wt[:], in_=w[:])
            else:
                nc.sync.dma_start(out=wt[:], in_=w[:])

            if variant == "big":
                # 6 super-tiles of 4096 cols + 1 of 2048 (fewer, larger DMAs)
                st_list = [(o * 4096, 4096) for o in range(6)] + [(24576, 2048)]
            elif variant in ("v2", "v2b", "v2bc"):
                # fine-grained first super-tile so compute starts ~2us earlier
                st_list = [(o * 512, 512) for o in range(4)] + [
                    (s * ST, ST) for s in range(1, N_ST)
                ]
            else:
                st_list = [(s * ST, ST) for s in range(N_ST)]

            def body():
                for off, stn in st_list:
                    mv = mvp.tile([K, stn], in_dt, tag="mv")
                    nc.sync.dma_start(out=mv[0:H, :], in_=xr_r[:, :, ds(off, stn)])
                    nc.sync.dma_start(out=mv[H:K, :], in_=xi_r[:, :, ds(off, stn)])
                    if variant == "dma":
                        # stream straight back out, skipping compute
                        nc.scalar.dma_start(
                            out=yre_r[:, :, ds(off, stn)], in_=mv[0:H, :].bitcast(f32)
                        )
                        nc.scalar.dma_start(
                            out=yim_r[:, :, ds(off, stn)], in_=mv[H:K, :].bitcast(f32)
                        )
                        continue
                    ot = op.tile([K, stn], f32, tag="ot")
                    for j in range(stn // TILE_N):
                        ps = pp.tile([K, TILE_N], f32, tag="ps")
                        nc.tensor.matmul(
                            out=ps[:],
                            lhsT=wt[:],
                            rhs=mv[:, ts(j, TILE_N)],
                            start=True,
                            stop=True,
                        )
                        if j % 2 == 0:
                            nc.vector.tensor_copy(out=ot[:, ts(j, TILE_N)], in_=ps[:])
                        else:
                            nc.scalar.copy(out=ot[:, ts(j, TILE_N)], in_=ps[:])
                    if variant == "nooutdma":
                        continue
                    # Output DMAs off the SP ring so they don't head-of-line-block
                    # the next tile's input DMAs. v4: SWDGE (idle Pool engine) so
                    # they don't block ACT's next-tile copies either.
                    odma = nc.gpsimd if variant == "v4" else nc.scalar
                    odma.dma_start(out=yre_r[:, :, ds(off, stn)], in_=ot[0:H, :])
                    odma.dma_start(out=yim_r[:, :, ds(off, stn)], in_=ot[H:K, :])

            if reps == 1:
                body()
            else:
                with tc.For_i(0, reps, 1):
                    body()

    nc.compile()
    _CACHE[key] = nc
    return nc


def _pad_shard(plane, sl):
    out = np.zeros((N_CH, B_PAD), np.float32)
    out[:, :B_CORE] = plane[:, sl]
    return out


def kernel(field_re, field_im, mzi_phases, output_phases):
    from concourse import bass_utils

    field_re = np.asarray(field_re)
    field_im = np.asarray(field_im)
    U = _build_unitary(mzi_phases, output_phases)
    W = _build_weights(U)

    # v4: output DMAs ride the SWDGE ring (idle GpSimd engine) — measured
    # faster and more stable than issuing them from the ACT sequencer, whose
    # instruction stream they would otherwise block.
    nc = _get_compiled(variant="v4")
    in_maps = []
    for i in range(N_CORES):
        sl = slice(i * B_CORE, (i + 1) * B_CORE)
        in_maps.append(
            {
                "xr": _pad_shard(field_re, sl),
                "xi": _pad_shard(field_im, sl),
                "w": W,
            }
        )
    res = bass_utils.run_bass_kernel_spmd(nc, in_maps, core_ids=list(range(N_CORES)))

    out = np.empty((N_CH, BATCH), np.complex64)
    for i in range(N_CORES):
        sl = slice(i * B_CORE, (i + 1) * B_CORE)
        out.real[:, sl] = res.results[i]["yre"][:, :B_CORE]
        out.imag[:, sl] = res.results[i]["yim"][:, :B_CORE]
    return out


# revision 30
# speedup vs baseline: 78.9169x; 1.0307x over previous
"""Trainium2 Bass kernel for nn_CmxuLayer: y = U.T @ X, U = 6x6 complex unitary
built from 36 phases, X = [6, 2097152] complex64 given as separate re/im f32 planes.

Strategy (pure data parallel over 8 NeuronCores):
  - Host builds the 6x6 unitary U from the phases (negligible), and packs it into a
    real [120, 120] stationary matrix W implementing the complex matmul on 10
    batch-groups at once (120 = 12 re/im channel components x 10 groups).
  - Each core gets a contiguous batch shard of 262144 columns, zero-padded to
    266240 and reshaped to 10 groups x 26624. The moving operand is [120, N] f32
    in SBUF: partitions 0..59 = re channels (c*10+g), 60..119 = im channels.
    120 partitions balance the SBUF<->DMA port swizzle (96 would leave the even
    SDMA engines carrying 2x the bytes of the odd ones).
  - One fp32 PE matmul per 512 columns -> PSUM [120, 512]; DVE/ACT copy to SBUF;
    DMA out as separate re/im planes. Host re-assembles complex64 on gather.
    (fp32 matmul streams at 4 cyc/col but still hides under the DMA floor;
    measured <1% slower than the reduced-precision fp32r mode.)
  - Each data stream gets its own issuing engine: input DMAs on the SP HWDGE
    ring, output DMAs on the GpSimd SWDGE ring, PSUM copies split across
    DVE/ACT — so no stream's stall can head-of-line-block another's issue.
"""

import numpy as np

N_CH = 6
BATCH = 2097152
N_CORES = 8
B_CORE = BATCH // N_CORES      # 262144 true columns per core
G = 10                         # batch groups per core (packed in partition dim)
NG = 26624                     # padded columns per group (13 * 2048)
B_PAD = G * NG                 # 266240 padded columns per core
K = 12 * G                     # 120 partitions
TILE_N = 512                   # matmul free dim (one PSUM bank @ fp32)
ST = 2048                      # per-group columns per super-tile (DMA granularity)
N_ST = NG // ST                # 13 super-tiles per core
USE_F32R = False               # fp32 native: full precision; PE hides under DMA anyway

_CACHE = {}


def _build_unitary(mzi_phases, output_phases):
    """Mirror reference.build_unitary in numpy (f32/c64 arithmetic)."""
    n = N_CH
    U = np.eye(n, dtype=np.complex64)
    idx = 0
    mz = np.asarray(mzi_phases, np.float32)
    op = np.asarray(output_phases, np.float32)
    j1 = np.complex64(1j)
    for i in range(n):
        for j in range(i + 1, n):
            theta = mz[idx]
            phi = mz[idx + 1]
            idx += 2
            c = np.complex64(np.cos(theta))
            s = np.complex64(np.sin(theta))
            eip = np.exp(j1 * phi).astype(np.complex64)
            row_i = eip * c * U[i] + s * U[j]
            row_j = -eip * s * U[i] + c * U[j]
            U = U.copy()
            U[i] = row_i
            U[j] = row_j
    U = np.exp(j1 * op)[:, None].astype(np.complex64) * U
    return U


def _build_weights(U):
    """Pack U into the [K, K] f32 stationary lhsT.

    matmul computes out[m, n] = sum_k lhsT[k, m] * rhs[k, n].
    rhs partition k = ci*G + g holds xr[ci] of group g (ci in 0..5),
                 k = (6+ci)*G + g holds xi[ci] of group g.
    out partition m = c*G + g is y_re[c] of group g,
                  m = (6+c)*G + g is y_im[c] of group g.
    y = U.T x  =>  y[c] = sum_ci U[ci, c] x[ci].
    """
    Ur = np.ascontiguousarray(U.real.astype(np.float32))
    Ui = np.ascontiguousarray(U.imag.astype(np.float32))
    W = np.zeros((K, K), np.float32)
    for g in range(G):
        for ci in range(N_CH):
            for c in range(N_CH):
                W[ci * G + g, c * G + g] = Ur[ci, c]
                W[(6 + ci) * G + g, c * G + g] = -Ui[ci, c]
                W[ci * G + g, (6 + c) * G + g] = Ui[ci, c]
                W[(6 + ci) * G + g, (6 + c) * G + g] = Ur[ci, c]
    return W


def _get_compiled(reps=1, variant="full", f32r=None):
    if f32r is None:
        f32r = USE_F32R
    key = ("nc", reps, variant, f32r)
    if key in _CACHE:
        return _CACHE[key]

    import concourse.bass as bass
    import concourse.mybir as mybir
    from concourse import bacc
    from concourse.bass import ds, ts
    from concourse.tile import TileContext

    f32 = mybir.dt.float32
    in_dt = mybir.dt.float32r if f32r else f32
    nc = bacc.Bacc(
        trn_type="TRN2",
        target_bir_lowering=False,
        debug=False,
        num_devices=N_CORES,
    )
    H = K // 2  # 60: partition split between re and im halves
    xr = nc.dram_tensor("xr", [N_CH, B_PAD], in_dt, kind="ExternalInput").ap()
    xi = nc.dram_tensor("xi", [N_CH, B_PAD], in_dt, kind="ExternalInput").ap()
    w = nc.dram_tensor("w", [K, K], in_dt, kind="ExternalInput").ap()
    yre = nc.dram_tensor("yre", [N_CH, B_PAD], f32, kind="ExternalOutput").ap()
    yim = nc.dram_tensor("yim", [N_CH, B_PAD], f32, kind="ExternalOutput").ap()

    xr_r = xr.rearrange("c (g n) -> c g n", g=G)
    xi_r = xi.rearrange("c (g n) -> c g n", g=G)
    yre_r = yre.rearrange("c (g n) -> c g n", g=G)
    yim_r = yim.rearrange("c (g n) -> c g n", g=G)

    n_bufs = {"v2": 6, "v2c": 6, "v2ac": 6, "v2bc": 6, "v3c": 8, "v3ac": 8}.get(
        variant, 4
    )
    with TileContext(nc) as tc:
        with (
            tc.tile_pool(name="wpool", bufs=1) as wp,
            tc.tile_pool(name="mv", bufs=n_bufs) as mvp,
            tc.tile_pool(name="ot", bufs=n_bufs) as op,
            tc.tile_pool(name="ps", bufs=8, space="PSUM") as pp,
        ):
            wt = wp.tile([K, K], in_dt)
            if variant in ("v2", "v2a", "v2ac", "v3ac"):
                # SWDGE (gpsimd) ring: keeps the 120 sub-512B weight
                # descriptors off the SP ring ahead of the first input DMAs.
                nc.gpsimd.dma_start(out=wt[:], in_=w[:])
            else:
                nc.sync.dma_start(out=wt[:], in_=w[:])

            if variant == "big":
                # 6 super-tiles of 4096 cols + 1 of 2048 (fewer, larger DMAs)
                st_list = [(o * 4096, 4096) for o in range(6)] + [(24576, 2048)]
            elif variant in ("v2", "v2b", "v2bc"):
                # fine-grained first super-tile so compute starts ~2us earlier
                st_list = [(o * 512, 512) for o in range(4)] + [
                    (s * ST, ST) for s in range(1, N_ST)
                ]
            else:
                st_list = [(s * ST, ST) for s in range(N_ST)]

            def body():
                for off, stn in st_list:
                    mv = mvp.tile([K, stn], in_dt, tag="mv")
                    nc.sync.dma_start(out=mv[0:H, :], in_=xr_r[:, :, ds(off, stn)])
                    nc.sync.dma_start(out=mv[H:K, :], in_=xi_r[:, :, ds(off, stn)])
                    if variant == "dma":
                        # stream straight back out, skipping compute
                        nc.scalar.dma_start(
                            out=yre_r[:, :, ds(off, stn)], in_=mv[0:H, :].bitcast(f32)
                        )
                        nc.scalar.dma_start(
                            out=yim_r[:, :, ds(off, stn)], in_=mv[H:K, :].bitcast(f32)
                        )
                        continue
                    ot = op.tile([K, stn], f32, tag="ot")
                    for j in range(stn // TILE_N):
                        ps = pp.tile([K, TILE_N], f32, tag="ps")
                        nc.tensor.matmul(
                            out=ps[:],
                            lhsT=wt[:],
                            rhs=mv[:, ts(j, TILE_N)],
                            start=True,
                            stop=True,
                        )
                        if j % 2 == 0:
                            nc.vector.tensor_copy(out=ot[:, ts(j, TILE_N)], in_=ps[:])
                        else:
                            nc.scalar.copy(out=ot[:, ts(j, TILE_N)], in_=ps[:])
                    if variant == "nooutdma":
                        continue
                    # Output DMAs off the SP ring so they don't head-of-line-block
                    # the next tile's input DMAs. v4: SWDGE (idle Pool engine) so
                    # they don't block ACT's next-tile copies either.
                    odma = nc.gpsimd if variant == "v4" else nc.scalar
                    odma.dma_start(out=yre_r[:, :, ds(off, stn)], in_=ot[0:H, :])
                    odma.dma_start(out=yim_r[:, :, ds(off, stn)], in_=ot[H:K, :])

            if reps == 1:
                body()
            else:
                with tc.For_i(0, reps, 1):
                    body()

    nc.compile()
    _CACHE[key] = nc
    return nc


def _pad_shard(plane, sl):
    out = np.zeros((N_CH, B_PAD), np.float32)
    out[:, :B_CORE] = plane[:, sl]
    return out


def kernel(field_re, field_im, mzi_phases, output_phases):
    from concourse import bass_utils

    field_re = np.asarray(field_re)
    field_im = np.asarray(field_im)
    U = _build_unitary(mzi_phases, output_phases)
    W = _build_weights(U)

    # v4: output DMAs ride the SWDGE ring (idle GpSimd engine) — measured
    # faster and more stable than issuing them from the ACT sequencer, whose
    # instruction stream they would otherwise block.
    nc = _get_compiled(variant="v4")
    in_maps = []
    for i in range(N_CORES):
        sl = slice(i * B_CORE, (i + 1) * B_CORE)
        in_maps.append(
            {
                "xr": _pad_shard(field_re, sl),
                "xi": _pad_shard(field_im, sl),
                "w": W,
            }
        )
    res = bass_utils.run_bass_kernel_spmd(nc, in_maps, core_ids=list(range(N_CORES)))

    out = np.empty((N_CH, BATCH), np.complex64)
    for i in range(N_CORES):
        sl = slice(i * B_CORE, (i + 1) * B_CORE)
        out.real[:, sl] = res.results[i]["yre"][:, :B_CORE]
        out.imag[:, sl] = res.results[i]["yim"][:, :B_CORE]
    return out

